# revision 22
# baseline (speedup 1.0000x reference)
"""Distributed multi-head causal attention for TRN2, 8 NeuronCores.

Strategy (tensor-parallel heads for attention + output-dim-parallel wo,
re-shard via 5 chunked AllGathers):
  - Each core owns 2 of the 16 heads. It computes Q,K,V projections for its
    heads over the full sequence (both batches), applies RoPE, and computes
    causal softmax(QK^T/sqrt(hd)) @ V for its heads.
  - Everything on-chip is laid out TRANSPOSED: qT/kT are [hd, B*S], scores are
    [k, q], attention output is [hd, q].  This avoids all transposes:
      scoresT = kT_block.T @ qT        (lhsT=kT block, rhs=qT)
      aoT     = v_block.T  @ pT        (lhsT=v natural [k,hd], rhs=pT [k,q])
    Softmax denominator (sum over k = partition axis) comes from a ones-matmul
    (lhsT=ones [128,128]) that also broadcasts the sum across partitions.
    exp() is computed WITHOUT max subtraction (max |score| ~ 6, safe in f32).
  - QKV projection matmuls amortize LDWEIGHTS: one weight chunk feeds up to
    3 N=512 matmuls into a multi-bank PSUM tile (bf16 streaming runs ~2
    cols/cycle, so the exposed LDWEIGHTS is the dominant per-MM cost).
  - Attention output chunks are AllGathered across the 8 cores (5
    collectives: 1024+1024+1024+512+512 positions, each overlapped with
    later attention / wo compute).  Each core computes its 256 OUTPUT DIMS
    of the wo projection with N=512 matmuls:
    outT[od, q] = wot_chunk.T @ gathered[ad, q].  wo chunks for early data
    are interleaved into the attention stream; only the last 512-position
    chunk's projection sits in the post-collective tail.
  - Compute dtype: bf16 matmul operands, f32 PSUM accumulation, f32 softmax.

Host-side prep casts inputs to bf16 and pre-transposes x/wo; host-side
assembly transposes/concats per-core outputs.  No host arithmetic.
"""
import math

import ml_dtypes
import numpy as np

import concourse.bass as bass
import concourse.mybir as mybir
from concourse import bacc
from concourse.tile import TileContext

F32 = mybir.dt.float32
BF16 = mybir.dt.bfloat16

N_CORES = 8
CORE_IDS = list(range(N_CORES))
B = 2
S = 2048
D = 2048
H = 16
HD = 128  # head dim
HPC = H // N_CORES  # heads per core = 2
BS = B * S  # 4096
NB = S // 512  # 4 q-free-blocks per batch
NK = S // 128  # 16 k-blocks per batch
ODIM = D // N_CORES  # 256 output dims per core
INV_SQRT_HD = 1.0 / math.sqrt(HD)

# AllGather chunks: (first 512-block, n 512-blocks) over the 8 (b,jq) blocks
AG_CHUNKS = [(0, 2), (2, 2), (4, 2), (6, 1), (7, 1)]
AG_OF_BLOCK = {}
for _a, (_s, _n) in enumerate(AG_CHUNKS):
    for _j in range(_s, _s + _n):
        AG_OF_BLOCK[_j] = (_a, _j - _s)

# qk projection LDW-sharing groups of 512-position chunks
QK_GROUPS = [(0,), (1, 2), (3, 4, 5), (6, 7)]

# stream_shuffle mask: swap adjacent partitions within each 32-group
PAIR_SWAP = [i ^ 1 for i in range(32)]


def build():
    nc = bacc.Bacc(None, num_devices=N_CORES)

    # x transposed, position-chunk-major: 8 chunks of 512 positions
    xt = nc.declare_dram_parameter("xt", [8, 128, 16, 512], BF16, isOutput=False)
    wqt = nc.declare_dram_parameter("wqt", [128, 16, HPC * HD], BF16, isOutput=False)
    wkt = nc.declare_dram_parameter("wkt", [128, 16, HPC * HD], BF16, isOutput=False)
    wvt = nc.declare_dram_parameter("wvt", [128, 16, HPC * HD], BF16, isOutput=False)
    # wo.T slice for this core's 256 output dims: [128, 16 ad-chunks, 256 od]
    wot = nc.declare_dram_parameter("wot", [128, 16, ODIM], BF16, isOutput=False)
    cgrid = nc.declare_dram_parameter("cgrid", [HD, S], BF16, isOutput=False)
    sgrid = nc.declare_dram_parameter("sgrid", [HD, S], BF16, isOutput=False)
    masks = nc.declare_dram_parameter("masks", [HD, 2, 1024], BF16, isOutput=False)
    # my 256 output dims for all B*S positions (chunk-major columns)
    out_ext = nc.declare_dram_parameter("out", [ODIM, BS], F32, isOutput=True)

    bnc_in = [
        nc.dram_tensor(f"bounce_in{a}", [HPC * HD, n * 512], BF16)
        for a, (s, n) in enumerate(AG_CHUNKS)
    ]
    bnc_out = [
        nc.dram_tensor(f"bounce_out{a}", [N_CORES, HPC * HD, n * 512], BF16,
                       addr_space="Shared")
        for a, (s, n) in enumerate(AG_CHUNKS)
    ]

    bar_in = nc.dram_tensor("bar_in", [1], F32)
    bar_out = nc.dram_tensor("bar_out", [N_CORES], F32, addr_space="Shared")

    with TileContext(nc) as tc:
        with (
            tc.tile_pool(name="persist", bufs=1) as persist,
            tc.tile_pool(name="tmp", bufs=4) as tmp,
            tc.tile_pool(name="rope", bufs=2) as rope,
            tc.tile_pool(name="opool", bufs=2) as opool,
        ):
            # ---------------- persistent SBUF tensors ----------------
            mask_sb = persist.tile([128, 2, 1024], BF16, tag="mask")
            ones_sb = persist.tile([128, 128], BF16, tag="ones")
            nc.vector.memset(ones_sb, 1.0)

            # qT/kT per head: [hd=128, BS] bf16 (post-RoPE).
            # v per head: [128, BS] bf16, chunk ik at cols [128*ik,128*(ik+1))
            # holding v rows (k) on partitions, hd on free.
            q_sb = [persist.tile([128, BS], BF16, tag=f"q{h}", name=f"q_sb{h}") for h in range(HPC)]
            k_sb = [persist.tile([128, BS], BF16, tag=f"k{h}", name=f"k_sb{h}") for h in range(HPC)]
            v_sb = [persist.tile([128, BS], BF16, tag=f"v{h}", name=f"v_sb{h}") for h in range(HPC)]

            # ---------------- phase 1: QKV projections + RoPE ----------------
            with (
                tc.tile_pool(name="p1w", bufs=1) as p1w,
                tc.tile_pool(name="xt_pool", bufs=5) as xt_pool,
                tc.tile_pool(name="p1psum", bufs=2, space="PSUM") as p1psum,
                tc.tile_pool(name="p1psumv", bufs=2, space="PSUM") as p1psumv,
            ):
                wq_sb = p1w.tile([128, 16, HPC * HD], BF16, tag="wq")
                wk_sb = p1w.tile([128, 16, HPC * HD], BF16, tag="wk")
                wv_sb = p1w.tile([128, 16, HPC * HD], BF16, tag="wv")
                cg_sb = p1w.tile([128, S], BF16, tag="cg")
                sg_sb = p1w.tile([128, S], BF16, tag="sg")
                nc.gpsimd.dma_start(out=wq_sb, in_=wqt[:, :, :])
                nc.gpsimd.dma_start(out=wk_sb, in_=wkt[:, :, :])
                nc.gpsimd.dma_start(out=wv_sb, in_=wvt[:, :, :])
                nc.gpsimd.dma_start(out=cg_sb, in_=cgrid[:, :])
                nc.gpsimd.dma_start(out=sg_sb, in_=sgrid[:, :])
                nc.gpsimd.dma_start(out=mask_sb, in_=masks[:, :, :])
                # dummy AllGather: absorbs cross-core NEFF-launch skew early,
                # so the later collectives see aligned peers
                nc.gpsimd.collective_compute(
                    "AllGather",
                    mybir.AluOpType.bypass,
                    replica_groups=[CORE_IDS],
                    ins=[bar_in[:]],
                    outs=[bar_out[:]],
                )

                xts = []  # 8 chunk tiles of [128, 16, 512], 5 rotating bufs
                for pc in range(8):
                    xt_sb = xt_pool.tile([128, 16, 512], BF16, tag="xt",
                                         name=f"xt_sb{pc}")
                    # every chunk split across both queues: fast in-order ramp
                    engs = (nc.sync, nc.scalar)
                    for half in range(2):
                        engs[half].dma_start(
                            out=xt_sb[:, half * 8:(half + 1) * 8, :],
                            in_=xt[pc, :, half * 8:(half + 1) * 8, :],
                        )
                    xts.append(xt_sb)

                for grp in QK_GROUPS:
                    ng = len(grp)
                    for kind, w in (("q", wq_sb), ("k", wk_sb)):
                        for h in range(HPC):
                            # one weight chunk feeds ng N=512 matmuls
                            p = p1psum.tile([128, 1536], F32, tag="qk")
                            for i in range(16):
                                wsl = w[:, i, h * HD:(h + 1) * HD]
                                for gi in range(ng):
                                    nc.tensor.matmul(
                                        p[:, gi * 512:(gi + 1) * 512],
                                        wsl, xts[grp[gi]][:, i, :],
                                        start=(i == 0), stop=(i == 15),
                                    )
                            dst = q_sb if kind == "q" else k_sb
                            for gi in range(ng):
                                pc = grp[gi]
                                b = pc // 4
                                poff = (pc % 4) * 512
                                gcol = slice(poff, poff + 512)
                                ocol = slice(b * S + poff, b * S + poff + 512)
                                psl = p[:, gi * 512:(gi + 1) * 512]
                                # RoPE: out = t*cos + pairswap(t)*sin_signed
                                m1 = rope.tile([128, 512], F32, tag="rope_m1")
                                nc.vector.tensor_mul(m1, psl, cg_sb[:, gcol])
                                sh = rope.tile([128, 512], F32, tag="rope_sh")
                                nc.vector.stream_shuffle(sh, psl, PAIR_SWAP)
                                nc.vector.tensor_mul(sh, sh, sg_sb[:, gcol])
                                nc.vector.tensor_add(dst[h][:, ocol], m1, sh)

                    # V for both heads: psum [s=128, 2*HD] accumulated over d_in
                    for pc in grp:
                        b = pc // 4
                        for s2 in range(4):
                            pv = p1psumv.tile([128, HPC * HD], F32, tag="v")
                            for i in range(16):
                                nc.tensor.matmul(
                                    pv,
                                    xts[pc][:, i, s2 * 128:(s2 + 1) * 128],
                                    wv_sb[:, i, :],
                                    start=(i == 0), stop=(i == 15),
                                )
                            sc = (pc % 4) * 4 + s2
                            ccol = slice((b * NK + sc) * 128, (b * NK + sc + 1) * 128)
                            for h in range(HPC):
                                nc.scalar.copy(
                                    out=v_sb[h][:, ccol], in_=pv[:, h * HD:(h + 1) * HD]
                                )

            # ---------------- phases 2+3 pools ----------------
            with (
                tc.tile_pool(name="p23", bufs=1) as p23,
                tc.tile_pool(name="gpool", bufs=4) as gpool,
                tc.tile_pool(name="ptile", bufs=6) as ptile,
                tc.tile_pool(name="p2psum", bufs=2, space="PSUM") as p2psum,
            ):
                wot_sb = p23.tile([128, 16, ODIM], BF16, tag="wot")
                nc.gpsimd.dma_start(out=wot_sb, in_=wot[:, :, :])

                def attention_chunk(b, jq):
                    """Attention for 512-q-block jq of batch b, both heads;
                    stores [256, 512] bf16 into its AllGather bounce slot."""
                    aidx, off = AG_OF_BLOCK[b * NB + jq]
                    acol = slice(off * 512, off * 512 + 512)
                    for h in range(HPC):
                        po = p2psum.tile([128, 512], F32, tag="pv", bufs=1)
                        pden = p2psum.tile([128, 512], F32, tag="den", bufs=1)
                        nkb = 4 * jq + 4  # causal: k-blocks 0..4jq+3
                        qcol = slice(b * S + jq * 512, b * S + (jq + 1) * 512)
                        # q-columns 256-511 only (high diagonal pair)
                        qcol_hi = slice(b * S + jq * 512 + 256, b * S + (jq + 1) * 512)
                        d_prev = None
                        for e in range(nkb // 2):  # k-block pairs
                            hi = e == 2 * jq + 1  # high diagonal pair:
                            # blocks 4jq+2/4jq+3 only reach q >= 256
                            w = 256 if hi else 512
                            # fixed-shape tile (shared tag with wo psum, 3
                            # bufs): deep rotation so scores run ahead of exp
                            psc = p2psum.tile([128, 1024], F32, tag="sc",
                                              name="psc", bufs=3)[:, 0:2 * w]
                            for u in range(2):
                                ik = 2 * e + u
                                nc.tensor.matmul(
                                    psc[:, u * w:(u + 1) * w],
                                    k_sb[h][:, b * S + ik * 128: b * S + (ik + 1) * 128],
                                    q_sb[h][:, qcol_hi if hi else qcol],
                                    start=True,
                                    stop=True,
                                )
                            p_sb = ptile.tile([128, 2 * w], BF16, tag="p", name="p_sb")
                            nc.scalar.activation(
                                out=p_sb,
                                in_=psc,
                                func=mybir.ActivationFunctionType.Exp,
                                scale=INV_SQRT_HD,
                            )
                            if e >= 2 * jq:  # diagonal pair: causal 0/1 mask
                                nc.vector.tensor_mul(
                                    p_sb, p_sb, mask_sb[:, e - 2 * jq, 0:2 * w]
                                )
                            for u in range(2):
                                ik = 2 * e + u
                                vcol = slice((b * NK + ik) * 128, (b * NK + ik + 1) * 128)
                                nc.tensor.matmul(
                                    po[:, 256:512] if hi else po,
                                    v_sb[h][:, vcol], p_sb[:, u * w:(u + 1) * w],
                                    start=(ik == 0), stop=(ik == nkb - 1),
                                    skip_group_check=True,
                                )
                            d_sb = tmp.tile([128, 512], BF16, tag="dpair")
                            nc.vector.tensor_add(
                                d_sb[:, 0:w], p_sb[:, 0:w], p_sb[:, w:2 * w]
                            )
                            if e < 2 * jq and e % 2 == 0:
                                d_prev = d_sb  # defer: pair up with next
                                continue
                            if e < 2 * jq:  # odd off-diagonal: fold 2 pairs
                                dd = tmp.tile([128, 512], BF16, tag="dquad")
                                nc.vector.tensor_add(dd, d_prev, d_sb)
                                d_sb = dd
                            nc.tensor.matmul(
                                pden[:, 256:512] if hi else pden,
                                ones_sb, d_sb[:, 0:w],
                                start=(e == (1 if jq > 0 else 0) and not hi),
                                stop=hi,
                                skip_group_check=True,
                            )
                        recip = tmp.tile([128, 512], F32, tag="recip")
                        nc.vector.reciprocal_approx_fast(out=recip, in_=pden)
                        ao = tmp.tile([128, 512], BF16, tag="ao")
                        nc.vector.tensor_mul(ao, po, recip)
                        nc.sync.dma_start(
                            out=bnc_in[aidx][h * HD:(h + 1) * HD, acol],
                            in_=ao,
                        )

                def fire_ag(aidx):
                    nc.gpsimd.collective_compute(
                        "AllGather",
                        mybir.AluOpType.bypass,
                        replica_groups=[CORE_IDS],
                        ins=[bnc_in[aidx][:, :]],
                        outs=[bnc_out[aidx][:, :, :]],
                    )

                def g_load(b, jq, engines=(nc.sync, nc.sync)):
                    """Prefetch the gathered [2048, 512] attn chunk into SBUF,
                    split across two DMA queues."""
                    aidx, off = AG_OF_BLOCK[b * NB + jq]
                    acol = slice(off * 512, off * 512 + 512)
                    g_sb = gpool.tile([128, 16, 512], BF16, tag="g")
                    src = bnc_out[aidx].rearrange(
                        "j (u p) n -> p (j u) n", p=128)[:, :, acol]
                    engines[0].dma_start(out=g_sb[:, 0:8, :], in_=src[:, 0:8, :])
                    engines[1].dma_start(out=g_sb[:, 8:16, :], in_=src[:, 8:16, :])
                    return g_sb

                def wo_mms(b, jq, g_sb):
                    """My 256 output dims of the wo projection for 512-position
                    chunk (b, jq): outT[od, q] = wot.T @ gathered attnT."""
                    for od in range(2):
                        pw = p2psum.tile([128, 1024], F32, tag="sc",
                                         name="pw", bufs=3)[:, 0:512]
                        for i in range(16):
                            nc.tensor.matmul(
                                pw,
                                wot_sb[:, i, od * 128:(od + 1) * 128],
                                g_sb[:, i, :],
                                start=(i == 0),
                                stop=(i == 15),
                            )
                        o_sb = opool.tile([128, 512], F32, tag=f"o{od}")
                        nc.vector.tensor_copy(out=o_sb, in_=pw)
                        nc.sync.dma_start(
                            out=out_ext[od * 128:(od + 1) * 128,
                                        (b * NB + jq) * 512:(b * NB + jq + 1) * 512],
                            in_=o_sb,
                        )

                # batch 0 attention; AGs fire per completed 1024-position half
                attention_chunk(0, 0)
                attention_chunk(0, 1)
                fire_ag(0)
                attention_chunk(0, 2)
                attention_chunk(0, 3)
                fire_ag(1)
                # batch 1 attention interleaved with batch-0 wo chunks
                attention_chunk(1, 0)
                attention_chunk(1, 1)
                fire_ag(2)
                g00 = g_load(0, 0)
                g01 = g_load(0, 1)
                wo_mms(0, 0, g00)
                wo_mms(0, 1, g01)
                attention_chunk(1, 2)
                fire_ag(3)
                g02 = g_load(0, 2)
                g03 = g_load(0, 3)
                wo_mms(0, 2, g02)
                wo_mms(0, 3, g03)
                attention_chunk(1, 3)
                fire_ag(4)
                # tail: all g prefetches first, then the matmuls
                g10 = g_load(1, 0)
                g11 = g_load(1, 1)
                g12 = g_load(1, 2)
                g13 = g_load(1, 3, engines=(nc.sync, nc.sync))
                wo_mms(1, 0, g10)
                wo_mms(1, 1, g11)
                wo_mms(1, 2, g12)
                wo_mms(1, 3, g13)

    nc.compile()
    return nc


def prep_inputs(x, freqs_cos, freqs_sin, wq, wk, wv, wo):
    """Host-side shard prep. Returns in_maps (list of 8 dicts)."""
    bf = ml_dtypes.bfloat16
    x = np.asarray(x, dtype=np.float32)
    xtf = x.reshape(BS, D).T.astype(bf)  # [D, BS]
    # chunk-major: [pc, p, chunk, n] -> fully sequential DMAs per 512-chunk
    xt = np.ascontiguousarray(
        xtf.reshape(16, 128, 8, 512).transpose(2, 1, 0, 3))
    cos = np.asarray(freqs_cos, np.float32)
    sin = np.asarray(freqs_sin, np.float32)
    cg = np.empty((HD, S), np.float32)
    sg = np.empty((HD, S), np.float32)
    cg[0::2] = cos.T
    cg[1::2] = cos.T
    sg[0::2] = -sin.T
    sg[1::2] = sin.T
    mk4 = np.zeros((4, HD, 512), np.float32)
    for t in range(4):
        kp = np.arange(HD)[:, None]
        qf = np.arange(512)[None, :]
        mk4[t] = (128 * t + kp <= qf).astype(np.float32)
    # mk[0]: low diagonal pair (blocks t0,t1) over full 512 q-cols;
    # mk[1][:, :512]: high pair (t2,t3) restricted to q-cols 256-511
    mk = np.zeros((2, HD, 1024), np.float32)
    mk[0][:, 0:512] = mk4[0]
    mk[0][:, 512:1024] = mk4[1]
    mk[1][:, 0:256] = mk4[2][:, 256:512]
    mk[1][:, 256:512] = mk4[3][:, 256:512]
    mk = np.ascontiguousarray(mk.astype(bf).transpose(1, 0, 2))

    woT = np.asarray(wo, np.float32).T.astype(bf)  # [ad, od]
    in_maps = []
    for c in range(N_CORES):
        rows = slice(c * HPC * HD, (c + 1) * HPC * HD)
        ods = slice(c * ODIM, (c + 1) * ODIM)
        in_maps.append({
            "xt": xt,
            "wqt": np.ascontiguousarray(np.asarray(wq, np.float32)[rows, :].T.astype(bf).reshape(16, 128, HPC * HD).transpose(1, 0, 2)),
            "wkt": np.ascontiguousarray(np.asarray(wk, np.float32)[rows, :].T.astype(bf).reshape(16, 128, HPC * HD).transpose(1, 0, 2)),
            "wvt": np.ascontiguousarray(np.asarray(wv, np.float32)[rows, :].T.astype(bf).reshape(16, 128, HPC * HD).transpose(1, 0, 2)),
            "wot": np.ascontiguousarray(woT[:, ods].reshape(16, 128, ODIM).transpose(1, 0, 2)),
            "cgrid": cg.astype(bf),
            "sgrid": sg.astype(bf),
            "masks": mk,
        })
    return in_maps


def assemble(results):
    out = np.empty((B, S, D), np.float32)
    for c in range(N_CORES):
        r = results[c]["out"]  # [ODIM, BS], cols (b*4+jq)*512 chunk-major
        ods = slice(c * ODIM, (c + 1) * ODIM)
        for b in range(B):
            out[b, :, ods] = r[:, b * S:(b + 1) * S].T
    return out


_NC_CACHE = []


def kernel(**inputs):
    """Full-input distributed attention on 8 TRN2 NeuronCores.

    Takes the unsharded inputs (x, freqs_cos, freqs_sin, wq, wk, wv, wo) as
    numpy float32 arrays, runs the SPMD bass kernel on cores 0-7, and
    returns the full [B, S, D] float32 output.
    """
    from concourse.bass_utils import run_bass_kernel_spmd

    if not _NC_CACHE:
        _NC_CACHE.append(build())
    nc = _NC_CACHE[0]
    in_maps = prep_inputs(
        x=inputs["x"],
        freqs_cos=inputs["freqs_cos"],
        freqs_sin=inputs["freqs_sin"],
        wq=inputs["wq"],
        wk=inputs["wk"],
        wv=inputs["wv"],
        wo=inputs["wo"],
    )
    res = run_bass_kernel_spmd(nc, in_maps, CORE_IDS, trace=False)
    return assemble(res.results)
